# revision 20
# baseline (speedup 1.0000x reference)
"""DSPA (dual-softmax sparse attention) Trainium2 kernel.

Math (reference):
  x1 = x.reshape(2, 64, 4096)                       # [b, c, n]
  x2 = dilated_conv1d(x1, w, b) (k=3, dil=2, pad=1) # [b, c, m], m = n-2
  s[b] = x1[b]^T @ x2[b]                            # [n, m]
  x3 = softmax over b (2 elements)  ->  x3[0] = sigmoid(s0 - s1), x3[1] = 1 - x3[0]
  x4[b] = x2[b] @ x3[b]^T                           # [c, n]
  out = x1 + x4

Key identities used:
  p := sigmoid(d),  d := s0 - s1
  x4[0] = x2[0] @ p^T
  x4[1] = S1 - x2[1] @ p^T          with S1[c] = sum_j x2[1][c, j]

Device strategy (8 cores, no collectives):
  - Shard the query dim n=4096 -> 512 columns per core.
  - Each core computes the full conv x2 on-chip (cheap) in a batch-stacked,
    batch1-NEGATED bf16 layout x2bf [128, 4096] so that ONE matmul with
    K=128 (c of batch0 stacked on c of batch1) yields d^T directly in PSUM.
  - p^T tiles ([j=128, i=512]) come from ACT sigmoid (bf16); stage-3
    matmuls contract over j using x2^T tiles produced by DMA-engine (xbar)
    transposes, accumulating x4 for both batches in one PSUM bank.

Everything is bf16 except the PSUM accumulations (fp32) and the final
epilogue; matmul weight loads get the FWL fast path and the transposes
run on otherwise-idle DMA engines.
"""

import numpy as np

import concourse.bacc as bacc
import concourse.mybir as mybir
import concourse.tile as tile
from concourse.bass_utils import run_bass_kernel_spmd

F32 = mybir.dt.float32
BF16 = mybir.dt.bfloat16

B, C, N, M = 2, 64, 4096, 4094
NCORES = 8
ISL = N // NCORES          # 512 query columns per core
NT = 32                    # j tiles of 128 (last = 126)
NCH = 8                    # conv chunks of 512 (last = 510)
MISC_W = 3 * 128 + 128 + 2 + ISL  # ident + 2 bf16 cols hold fp32 bias bits


def build_nc():
    nc = bacc.Bacc()

    xin = nc.dram_tensor("xin", [128, N + 2], BF16, kind="ExternalInput")
    misc = nc.dram_tensor("misc", [128, MISC_W], BF16, kind="ExternalInput")
    out = nc.dram_tensor("out", [128, ISL], F32, kind="ExternalOutput")

    with tile.TileContext(nc) as tc:
        with (
            tc.tile_pool(name="persist", bufs=1) as persist,
            tc.tile_pool(name="ptiles", bufs=4) as ptiles,
            tc.tile_pool(name="psum_acc", bufs=1, space="PSUM") as psum_acc,
        ):
            x1pads = persist.tile([128, N + 2], BF16)
            msb = persist.tile([128, MISC_W], BF16)
            x2bf = persist.tile([128, NT * 128], BF16)
            x2T2bf = persist.tile([128, NT * 128], BF16)
            scratch = persist.tile([128, 1], F32)
            warm_sb = persist.tile([128, 512], F32)
            s1corr = persist.tile([128, 1], F32)
            s1parts = persist.tile([128, NCH], F32)
            osb = persist.tile([128, ISL], F32)

            def wsb(k):
                return msb[:, 128 * k : 128 * (k + 1)]

            isb = msb[:, 384:512]
            bsb = msb[:, 512:514].bitcast(F32)
            x1q = msb[:, 514 : 514 + ISL]

            # Preload the sigmoid ACT table set while DMAs run.
            nc.vector.memset(scratch[:, :], 0.0)
            nc.scalar.activation(
                scratch[:, :], scratch[:, :], mybir.ActivationFunctionType.Sigmoid
            )

            # Warm the PE (HAM clock gate) while the input DMAs run:
            # fp32 matmuls are 4 cycles/row, so 3 give ~4us of warm-up.
            nc.vector.memset(warm_sb[:, :], 0.0)
            nc.vector.memset(x2bf[:, M : NT * 128], 0.0)
            with tc.tile_pool(name="psum_warm", bufs=1, space="PSUM") as psum_warm:
                wps = psum_warm.tile([128, 512], F32)
                for i in range(2):
                    nc.tensor.matmul(wps[:, :], warm_sb[:, 0:128], warm_sb[:, :])

            # Input loads. xin arrives host-padded ([0, x1, 0]).
            nc.sync.dma_start(msb[:, :], misc[:, :])
            # Small first chunk: its completion semaphore fires early so the
            # first conv chunk (cols 0..515) starts ~1.5us sooner.
            CUTS = [0, 520, 1720, 2920, N + 2]
            for q in range(4):
                c0, c1 = CUTS[q], CUTS[q + 1]
                nc.sync.dma_start(x1pads[:, c0:c1], xin[:, c0:c1])

            # Conv chunks INTERLEAVED with the attention main loop: after
            # chunk ch the j-tiles 4ch..4ch+3 are ready (x2bf + x2T2bf), so
            # groups {2ch, 2ch+1} (two j-tiles each) run immediately. This
            # keeps ACT busy from the first chunk and the PE dense (HAM
            # stays warm).
            #
            # Conv: x2bf[:, j] = sum_k wblkT[k].T @ x1pads[:, j + 2k] (+bias)
            # wblkT is block-diagonal with the batch-1 block negated, so the
            # bottom 64 partitions hold -x2[1]. The conv evacuation also
            # emits per-chunk row sums (for the S1 correction) for free.
            acc = psum_acc.tile([128, ISL], F32)
            with (
                tc.tile_pool(name="psum_conv", bufs=2, space="PSUM") as psum_conv,
                tc.tile_pool(name="psum_tr", bufs=1, space="PSUM") as psum_tr,
                tc.tile_pool(name="psum_d", bufs=2, space="PSUM") as psum_d,
            ):
                for ch in range(NCH):
                    j0 = ch * 512
                    w = min(512, M - j0)
                    pc = psum_conv.tile([128, 512], F32, name=f"pc{ch}", tag="pc")
                    for k in range(3):
                        nc.tensor.matmul(
                            pc[:, 0:w],
                            wsb(k),
                            x1pads[:, j0 + 2 * k : j0 + 2 * k + w],
                            start=(k == 0),
                            stop=(k == 2),
                        )
                    nc.vector.tensor_scalar(
                        x2bf[:, j0 : j0 + w],
                        pc[:, 0:w],
                        bsb,
                        0.0,
                        op0=mybir.AluOpType.add,
                        op1=mybir.AluOpType.add,
                        accum_out=s1parts[:, ch : ch + 1],
                    )
                    # x2^T tiles via PE transpose (1 cycle/row at bf16) +
                    # DVE evacuation (4x-mode bf16 copies).
                    for tt in range(4):
                        t = 4 * ch + tt
                        tr = psum_tr.tile([128, 128], BF16, name=f"tr{t}", tag="tr")
                        nc.tensor.transpose(
                            tr[:, :], x2bf[:, 128 * t : 128 * (t + 1)], isb
                        )
                        nc.vector.tensor_copy(
                            x2T2bf[:, 128 * t : 128 * (t + 1)], tr[:, :]
                        )
                    if ch == NCH - 1:
                        # S1 correction vector: zeros on top, +S1[c] on the
                        # bottom (negated sum of -x2[1] partial row sums).
                        nc.vector.memset(s1corr[0:64, 0:1], 0.0)
                        nc.vector.reduce_sum(
                            s1corr[64:128, 0:1],
                            s1parts[64:128, :],
                            axis=mybir.AxisListType.X,
                            negate=True,
                        )
                    for gg in (2 * ch, 2 * ch + 1):
                        ts = [2 * gg, 2 * gg + 1]
                        d = psum_d.tile([128, 1024], F32, name=f"d{gg}", tag="d")
                        for u, t in enumerate(ts):
                            jt = t * 128
                            wt = min(128, M - jt)
                            # d^T tile: [j, i] = (s0 - s1)^T for this slice
                            nc.tensor.matmul(
                                d[0:wt, 512 * u : 512 * u + 512],
                                x2bf[:, jt : jt + wt],
                                x1q,
                                start=True,
                                stop=True,
                            )
                        p = ptiles.tile([128, 1024], BF16, name=f"p{gg}", tag="p")
                        nc.scalar.activation(
                            p[:, :], d[:, :], mybir.ActivationFunctionType.Sigmoid
                        )
                        for u, t in enumerate(ts):
                            jt = t * 128
                            # K=128 always: tile 31's two missing j rows are
                            # zero in x2T2bf, so stale p rows contribute 0.
                            nc.tensor.matmul(
                                acc[:, :],
                                x2T2bf[:, jt : jt + 128],
                                p[:, 512 * u : 512 * u + 512],
                                start=(t == 0),
                                stop=(t == NT - 1),
                            )

            # Epilogue: out = (acc + S1corr) + x1 in one DVE pass
            # (acc bottom holds -x2[1] @ p^T, so adding S1 gives x4[1]).
            nc.vector.scalar_tensor_tensor(
                osb[:, :],
                acc[:, :],
                s1corr[:, 0:1],
                x1q,
                op0=mybir.AluOpType.add,
                op1=mybir.AluOpType.add,
            )
            nc.sync.dma_start(out[:, :], osb[:, :])

    nc.finalize()
    return nc


_NC_CACHE = None


def _get_nc():
    global _NC_CACHE
    if _NC_CACHE is None:
        _NC_CACHE = build_nc()
    return _NC_CACHE


def _host_prep(x, conv_w, conv_b):
    import ml_dtypes

    x1 = np.zeros((B * C, N + 2), dtype=np.float32)
    x1[:, 1 : N + 1] = x.reshape(B * C, N)
    x1 = x1.astype(ml_dtypes.bfloat16)
    misc = np.zeros((128, MISC_W), dtype=np.float32)
    for k in range(3):
        wT = conv_w[:, :, k].T.astype(np.float32)  # [i, o]
        misc[0:64, 128 * k : 128 * k + 64] = wT
        misc[64:128, 128 * k + 64 : 128 * k + 128] = -wT
    misc[:, 384:512] = np.eye(128, dtype=np.float32)
    misc = misc.astype(ml_dtypes.bfloat16)
    bias = np.concatenate([conv_b, -conv_b]).astype(np.float32).reshape(128, 1)
    misc[:, 512:514] = bias.view(np.uint32).view(ml_dtypes.bfloat16).reshape(128, 2)
    return x1, misc


def kernel(x, conv_w, conv_b, _trace=False):
    x = np.asarray(x)
    conv_w = np.asarray(conv_w)
    conv_b = np.asarray(conv_b)
    x1, misc = _host_prep(x, conv_w, conv_b)

    in_maps = []
    for r in range(NCORES):
        mr = misc.copy()
        mr[:, 514:] = x1[:, 1 + r * ISL : 1 + (r + 1) * ISL]
        in_maps.append({"xin": x1, "misc": mr})

    nc = _get_nc()
    res = run_bass_kernel_spmd(nc, in_maps, list(range(NCORES)), trace=_trace)
    out = np.concatenate([res.results[r]["out"] for r in range(NCORES)], axis=1)
    out = out.reshape(B, C, 16, 16, 16).astype(np.float32)
    if _trace:
        return out, res
    return out
